# revision 6
# baseline (speedup 1.0000x reference)
"""GQA attention (B=2,S=2048,E=2048,H=16,KV=4,D=128, RoPE, causal) on 8 trn2 cores.

Sharding: core c = (b = c//4, kv = c%4). Tensor-parallel over kv-head groups
(Wq cols / Wk,Wv cols / Wo rows) x data-parallel over batch. Each core computes
a full [S, E] partial output (its head group's contribution) in bf16; host sums
the 4 partials per batch element in f32.

v2 schedule (from perfetto analysis of v1 @ 311us):
  - All inputs host-packed into the exact SBUF layout -> one contiguous DMA
    per weight tensor / per x block (v1 issued ~124 strided DMAs at ~650ns
    queue cost each; startup was DMA-gated for ~40us with the PE HAM-cold).
  - Softmax reciprocal on DVE (reciprocal_approx_fast) instead of ACT
    exp(-ln(x)): kills all ACT table swaps (11 x 1.3us) and the rsv copies.
  - All PSUM evacuations (rope raw, v, oraw) on DVE; ACT does exps only.
    yproj casts on ACT except block 2 (they land in the ACT-bound window 3).
  - In-order PE queue => stalls can only be filled by *emission-order*
    interleaving: next-block projections AND the PREVIOUS block's output
    projection are emitted as closures popped between attention strips.
    yproj(j) runs inside window(j+1); yproj(3) is the tail (pure PE, keeps
    HAM warm through the end).
  - PSUM: psS 2 (score strips) + psP 2 (proj accums/rot) + psO 2 (AV) +
    psY 2 (rowsum/broadcast/yproj/warmup) = 8 banks.
"""
import sys
sys.path.insert(0, "/opt/trn_rl_repo")
import numpy as np
import ml_dtypes

BF = ml_dtypes.bfloat16

B, S, E = 2, 2048, 2048
H, KV, D = 16, 4, 128
G = H // KV          # 4 q heads per kv head / core
THETA = 10000.0
P = 128
NE = E // P          # 16 e-chunks
NB = 4               # s-blocks per core loop
BS = S // NB         # 512
NSC = S // P         # 16 s-chunks

_CACHE = {}


def _build():
    if "nc" in _CACHE:
        return _CACHE["nc"]
    import concourse.bass as bass
    import concourse.tile as tile
    from concourse import mybir, bacc

    f32 = mybir.dt.float32
    bf16 = mybir.dt.bfloat16
    EXP = mybir.ActivationFunctionType.Exp
    SCALE = 1.0 / np.sqrt(D)

    nc = bacc.Bacc("TRN2", target_bir_lowering=False, debug=False)
    xt_d = nc.declare_dram_parameter("xt", [P, NE * S], bf16, isOutput=False)
    wq_d = nc.declare_dram_parameter("wq", [P, NE * G * D], bf16, isOutput=False)
    wk_d = nc.declare_dram_parameter("wk", [P, NE * D], bf16, isOutput=False)
    wv_d = nc.declare_dram_parameter("wv", [P, NE * D], bf16, isOutput=False)
    wo_d = nc.declare_dram_parameter("wo", [P, G * E], bf16, isOutput=False)
    cos_d = nc.declare_dram_parameter("cosT", [P, S], bf16, isOutput=False)
    sin_d = nc.declare_dram_parameter("sinT", [P, S], bf16, isOutput=False)
    tri_d = nc.declare_dram_parameter("tri", [P, P], bf16, isOutput=False)
    perm_d = nc.declare_dram_parameter("perm", [P, P], bf16, isOutput=False)
    y_d = nc.declare_dram_parameter("y", [S, E], bf16, isOutput=True)

    with tile.TileContext(nc) as tc, \
         nc.allow_low_precision(reason="bf16 matmul pipeline"):
        import contextlib
        with contextlib.ExitStack() as ctx:
            cst = ctx.enter_context(tc.tile_pool(name="cst", bufs=1))
            xtp = ctx.enter_context(tc.tile_pool(name="xtp", bufs=3))
            vp = ctx.enter_context(tc.tile_pool(name="vp", bufs=16))
            kvp = ctx.enter_context(tc.tile_pool(name="kvp", bufs=1))
            qtp = ctx.enter_context(tc.tile_pool(name="qtp", bufs=12))
            rawp = ctx.enter_context(tc.tile_pool(name="rawp", bufs=4))
            rtp = ctx.enter_context(tc.tile_pool(name="rtp", bufs=8))
            exp_p = ctx.enter_context(tc.tile_pool(name="exp", bufs=8))
            esp = ctx.enter_context(tc.tile_pool(name="esp", bufs=2))
            recp = ctx.enter_context(tc.tile_pool(name="recp", bufs=8))
            otp = ctx.enter_context(tc.tile_pool(name="otp", bufs=16))
            ybp = ctx.enter_context(tc.tile_pool(name="ybp", bufs=3))
            psS = ctx.enter_context(tc.tile_pool(name="psS", bufs=2, space="PSUM"))
            psP = ctx.enter_context(tc.tile_pool(name="psP", bufs=2, space="PSUM"))
            psO = ctx.enter_context(tc.tile_pool(name="psO", bufs=2, space="PSUM"))
            psY = ctx.enter_context(tc.tile_pool(name="psY", bufs=2, space="PSUM"))

            # ---- HAM warmup: dependency-free matmuls keep the PE clock-gate
            # busy while the first DMAs land ----
            wz = cst.tile([P, P], bf16, tag="wz")
            nc.vector.memset(wz[:], 0.0)
            wps = psY.tile([P, BS], f32, tag="y")
            for _ in range(112):
                nc.tensor.matmul(wps[:, :P], wz[:], wz[:], start=True,
                                 stop=True)

            # ---- constants / weights (resident, one DMA each) ----
            cos_sb = cst.tile([P, S], bf16, tag="cos")
            sin_sb = cst.tile([P, S], bf16, tag="sin")
            tri_sb = cst.tile([P, P], bf16, tag="tri")
            perm_sb = cst.tile([P, P], bf16, tag="perm")
            ones_col = cst.tile([P, 1], bf16, tag="onc")
            nc.vector.memset(ones_col[:], 1.0)
            ones_row = cst.tile([1, P], bf16, tag="onr")
            nc.vector.memset(ones_row[:], 1.0)

            wk_sb = cst.tile([P, NE * D], bf16, tag="wk")
            wq_sb = cst.tile([P, NE * G * D], bf16, tag="wq")
            wv_sb = cst.tile([P, NE * D], bf16, tag="wv")
            wo_sb = cst.tile([P, G * E], bf16, tag="wo")

            # first-needed first. gpsimd: wk,wv,wo; scalar: wq,consts;
            # sync: xt block 0 (split in 4 so the K accum can start early).
            HQ = NE * G * D // 2
            nc.gpsimd.dma_start(wk_sb[:], wk_d[:, :])
            nc.scalar.dma_start(wq_sb[:, :HQ], wq_d[:, 0:HQ])
            nc.gpsimd.dma_start(wq_sb[:, HQ:], wq_d[:, HQ:2 * HQ])
            xt_src = xt_d.rearrange("p (e s) -> p e s", s=S)
            xt_tiles = {}
            xt0 = xtp.tile([P, NE * BS], bf16, tag="xt", name="xt0")
            xt0v = xt0.rearrange("p (e s) -> p e s", s=BS)
            for q in range(4):
                nc.sync.dma_start(xt0v[:, 4 * q:4 * q + 4, :],
                                  xt_src[:, 4 * q:4 * q + 4, 0:BS])
            xt_tiles[0] = xt0
            nc.scalar.dma_start(cos_sb[:], cos_d[:])
            nc.scalar.dma_start(sin_sb[:], sin_d[:])
            nc.gpsimd.dma_start(wv_sb[:], wv_d[:, :])
            nc.scalar.dma_start(tri_sb[:], tri_d[:])
            nc.scalar.dma_start(perm_sb[:], perm_d[:])
            nc.gpsimd.dma_start(wo_sb[:], wo_d[:, :])

            def load_xt(j):
                t = xtp.tile([P, NE * BS], bf16, tag="xt", name=f"xt{j}")
                tv = t.rearrange("p (e s) -> p e s", s=BS)
                nc.sync.dma_start(tv[:, :, :],
                                  xt_src[:, :, j * BS:(j + 1) * BS])
                xt_tiles[j] = t

            load_xt(1)

            kT_sb = kvp.tile([P, S], bf16, tag="kT")   # one kv head
            v_sb = [vp.tile([P, D], bf16, tag="v", name=f"v{i}")
                    for i in range(NSC)]

            def xt_chunk(j, e):
                return xt_tiles[j][:, e * BS:(e + 1) * BS]

            def rope_evac(dst, ps, j):
                """dst (bf16) = rope(ps) at abs position j*BS.

                ps: [d, BS] f32 PSUM projection. DVE evacuates to bf16, one
                PE perm-matmul for rotate-half, DVE combines with cos/sin."""
                raw = rawp.tile([P, BS], bf16, tag="raw", name="raw")
                nc.vector.tensor_copy(raw[:], ps[:])
                rot = psP.tile([P, BS], f32, tag="p", name="rot")
                nc.tensor.matmul(rot[:], perm_sb[:], raw[:],
                                 start=True, stop=True)
                cs = cos_sb[:, j * BS:(j + 1) * BS]
                sn = sin_sb[:, j * BS:(j + 1) * BS]
                tm = rtp.tile([P, BS], bf16, tag="rt", name="tm")
                nc.vector.tensor_mul(tm[:], raw[:], cs)
                t2 = rtp.tile([P, BS], bf16, tag="rt", name="t2")
                nc.vector.tensor_mul(t2[:], rot[:], sn)
                nc.vector.tensor_add(dst, tm[:], t2[:])

            def proj_closures(j):
                """Emission closures for block j's K/Q/V projections."""
                cls = []
                js = slice(j * BS, (j + 1) * BS)

                # K: 16 accum matmuls + rope
                kps = []  # holds the psum tile across closures

                def k_mm(e):
                    def f():
                        if e == 0:
                            kps.append(psP.tile([P, BS], f32, tag="p", name="kps"))
                        nc.tensor.matmul(kps[0][:],
                                         wk_sb[:, e * D:(e + 1) * D],
                                         xt_chunk(j, e),
                                         start=(e == 0), stop=(e == NE - 1))
                    return f
                for e in range(NE):
                    cls.append(k_mm(e))
                cls.append(lambda: rope_evac(kT_sb[:, js], kps[0], j))

                # Q: 4 heads x 16 accum matmuls + rope each
                qT = [None] * G
                qps = {}

                def q_mm(h, e):
                    def f():
                        if e == 0:
                            qps[h] = psP.tile([P, BS], f32, tag="p", name="qps")
                        nc.tensor.matmul(
                            qps[h][:],
                            wq_sb[:, e * G * D + h * D:e * G * D + (h + 1) * D],
                            xt_chunk(j, e),
                            start=(e == 0), stop=(e == NE - 1))
                    return f

                def q_rope(h):
                    def f():
                        qh = qtp.tile([P, BS], bf16, tag="qT", name="qh")
                        rope_evac(qh[:], qps[h], j)
                        qT[h] = qh
                    return f
                for h in range(G):
                    for e in range(NE):
                        cls.append(q_mm(h, e))
                    cls.append(q_rope(h))

                # V: 4 s-chunks x 16 accum matmuls + DVE evac each
                vps = {}

                def v_mm(sc, e):
                    def f():
                        if e == 0:
                            vps[sc] = psP.tile([P, D], f32, tag="p", name="vps")
                        nc.tensor.matmul(
                            vps[sc][:],
                            xt_tiles[j][:, e * BS + sc * P:e * BS + (sc + 1) * P],
                            wv_sb[:, e * D:(e + 1) * D],
                            start=(e == 0), stop=(e == NE - 1))
                    return f

                def v_evac(sc):
                    def f():
                        nc.vector.tensor_copy(v_sb[4 * j + sc][:], vps[sc][:])
                    return f
                for sc in range(4):
                    for e in range(NE):
                        cls.append(v_mm(sc, e))
                    cls.append(v_evac(sc))
                return cls, qT

            def yproj_closures(j, outT, cast_eng, dma_engs, chunk_dma=False):
                """Emission closures for block j's output projection.
                outT: list of 4 normalized [P, BS] bf16 tiles. chunk_dma
                issues a [P, BS] DMA right after each cast (fast drain for
                the kernel tail); otherwise one [P, E] DMA per sc row."""
                cls = []
                for sc in range(4):
                    yb = [None]

                    def ymm(sc, eb, yb):
                        def f():
                            if eb == 0:
                                yb[0] = ybp.tile([P, E], bf16, tag="y",
                                                 name="yb")
                            ypn = psY.tile([P, BS], f32, tag="y", name="ypn")
                            for h in range(G):
                                nc.tensor.matmul(
                                    ypn[:],
                                    outT[h][:, sc * P:(sc + 1) * P],
                                    wo_sb[:, h * E + eb * BS:h * E + (eb + 1) * BS],
                                    start=(h == 0), stop=(h == G - 1))
                            if cast_eng == "act":
                                nc.scalar.copy(
                                    yb[0][:, eb * BS:(eb + 1) * BS], ypn[:])
                            else:
                                nc.vector.tensor_copy(
                                    yb[0][:, eb * BS:(eb + 1) * BS], ypn[:])
                            if chunk_dma:
                                r0 = j * BS + sc * P
                                eng = dma_engs[(4 * sc + eb) % len(dma_engs)]
                                eng.dma_start(
                                    y_d[r0:r0 + P, eb * BS:(eb + 1) * BS],
                                    yb[0][:, eb * BS:(eb + 1) * BS])
                        return f
                    for eb in range(4):
                        cls.append(ymm(sc, eb, yb))

                    if not chunk_dma:
                        def ydma(sc, yb):
                            def f():
                                r0 = j * BS + sc * P
                                eng = dma_engs[sc % len(dma_engs)]
                                eng.dma_start(y_d[r0:r0 + P, :], yb[0][:])
                            return f
                        cls.append(ydma(sc, yb))
                return cls

            # ---- prologue: block 0 projections, straight emission ----
            cls0, qT0 = proj_closures(0)
            for f in cls0:
                f()
            qT_all = {0: qT0}

            outT_all = {}

            def attention_window(j, fillers):
                """Emit attention(j); pop filler closures between strips to
                keep the in-order PE queue stocked. Returns with normalize(j)
                appended (and flushed) so outT_all[j] is populated."""
                nt = 4 * j + 4
                qT = qT_all[j]
                outT_all[j] = [None] * G
                state = {"left": G * nt}

                def pop_fillers():
                    if not fillers:
                        return
                    k = -(-len(fillers) // max(state["left"], 1))
                    for _ in range(min(k, len(fillers))):
                        fillers.pop(0)()

                for h in range(G):
                    outp = psO.tile([P, BS], f32, tag="o", name="outp")
                    exs = esp.tile([P, BS], bf16, tag="es", name="exs")
                    for t in range(nt):
                        off = (t - 4 * j) * P if t >= 4 * j else 0
                        sp = psS.tile([P, BS], f32, tag="s", name="sp")
                        nc.tensor.matmul(sp[:, off:],
                                         kT_sb[:, t * P:(t + 1) * P],
                                         qT[h][:, off:], start=True, stop=True)
                        ex = exs if t == 0 else exp_p.tile([P, BS], bf16,
                                                           tag="ex", name="ex")
                        nc.scalar.activation(ex[:, off:], sp[:, off:], EXP,
                                             scale=SCALE)
                        if t >= 4 * j:
                            nc.vector.tensor_mul(ex[:, off:off + P],
                                                 ex[:, off:off + P], tri_sb[:])
                        if t > 0:
                            nc.vector.tensor_add(exs[:, off:], exs[:, off:],
                                                 ex[:, off:])
                        nc.tensor.matmul(outp[:, off:], v_sb[t][:],
                                         ex[:, off:],
                                         start=(t == 0), stop=(t == nt - 1),
                                         skip_group_check=(off > 0))
                        state["left"] -= 1
                        pop_fillers()
                    # rowsum -> 1/x on DVE; normalize deferred to a closure
                    rs = psY.tile([1, BS], f32, tag="y", name="rs")
                    nc.tensor.matmul(rs[:], ones_col[:], exs[:],
                                     start=True, stop=True)
                    rcp = recp.tile([1, BS], f32, tag="rcp", name="rcp")
                    nc.vector.reciprocal_approx_fast(rcp[:], rs[:])
                    recb = recp.tile([1, BS], bf16, tag="rec", name="recb")
                    nc.vector.tensor_copy(recb[:], rcp[:])
                    orw = otp.tile([P, BS], bf16, tag="orw", name="orw")
                    nc.vector.tensor_copy(orw[:], outp[:])

                    def norm(j, h, recb, orw):
                        def f():
                            rb = psY.tile([P, BS], f32, tag="y", name="rb")
                            nc.tensor.matmul(rb[:], ones_row[:], recb[:],
                                             start=True, stop=True)
                            ot = otp.tile([P, BS], bf16, tag="oT", name="ot")
                            nc.vector.tensor_mul(ot[:], orw[:], rb[:])
                            outT_all[j][h] = ot
                        return f
                    fillers.append(norm(j, h, recb, orw))

                while fillers:
                    fillers.pop(0)()

            # Window order 0,1,3,2: the last window is PE-bound (attention 2
            # + yproj 3) instead of the ACT-bound attention 3, so the PE
            # never starves (and HAM never re-throttles) before the tail.
            # w0: proj(1) fills.  w1: proj(2)+proj(3)+yproj(0).
            # w3: yproj(1).  w2: yproj(3).  tail: yproj(2).
            load_xt(2)
            pc1, qT_all[1] = proj_closures(1)
            attention_window(0, pc1)

            load_xt(3)
            pc2, qT_all[2] = proj_closures(2)
            pc3, qT_all[3] = proj_closures(3)
            attention_window(1, pc2 + pc3 + yproj_closures(
                0, outT_all[0], cast_eng="act", dma_engs=[nc.gpsimd]))

            attention_window(3, yproj_closures(
                1, outT_all[1], cast_eng="dve", dma_engs=[nc.gpsimd]))

            attention_window(2, yproj_closures(
                3, outT_all[3], cast_eng="dve", dma_engs=[nc.gpsimd]))

            # ---- tail: block 2 output projection (pure PE, HAM stays warm)
            for f in yproj_closures(2, outT_all[2], cast_eng="act",
                                    dma_engs=[nc.gpsimd, nc.sync],
                                    chunk_dma=True):
                f()

    nc.compile()
    _CACHE["nc"] = nc
    return nc


def _tables():
    inv = 1.0 / THETA ** (np.arange(0, D, 2, dtype=np.float64) / D)   # [64]
    t = np.arange(S, dtype=np.float64)
    fr = np.outer(inv, t)                    # [64, S]
    cosT = np.empty((P, S), dtype=np.float32)
    cosT[0:64] = np.cos(fr)
    cosT[64:128] = np.cos(fr)
    sinT = np.empty((P, S), dtype=np.float32)
    sinT[0:64] = np.sin(fr)
    sinT[64:128] = np.sin(fr)
    # tri[p, c] = 1 if p <= c (valid) else 0 — the causal boundary block
    tri = (np.arange(P)[:, None] <= np.arange(P)[None, :]).astype(np.float32)
    # perm as lhsT: rot = perm.T @ q -> rot[i] = -q[i+64] (i<64), q[i-64] (i>=64)
    perm = np.zeros((P, P), dtype=np.float32)
    perm[np.arange(64) + 64, np.arange(64)] = -1.0
    perm[np.arange(64), np.arange(64) + 64] = 1.0
    return cosT.astype(BF), sinT.astype(BF), tri.astype(BF), perm.astype(BF)


def _pack_rows(a):
    """[R*128, C] -> [128, R*C] with row r*128+p at packed[p, r*C:(r+1)*C]."""
    r = a.shape[0] // P
    return np.ascontiguousarray(
        a.reshape(r, P, a.shape[1]).transpose(1, 0, 2).reshape(P, -1))


def _in_maps(x, Wq, Wk, Wv, Wo):
    cosT, sinT, tri, perm = _tables()
    xt = [_pack_rows(np.ascontiguousarray(x[b].T).astype(BF))
          for b in range(B)]
    wq = [_pack_rows(np.ascontiguousarray(
        Wq[:, kv * G * D:(kv + 1) * G * D]).astype(BF)) for kv in range(KV)]
    wk = [_pack_rows(np.ascontiguousarray(
        Wk[:, kv * D:(kv + 1) * D]).astype(BF)) for kv in range(KV)]
    wv = [_pack_rows(np.ascontiguousarray(
        Wv[:, kv * D:(kv + 1) * D]).astype(BF)) for kv in range(KV)]
    wo = [_pack_rows(np.ascontiguousarray(
        Wo[kv * G * D:(kv + 1) * G * D, :]).astype(BF)) for kv in range(KV)]
    maps = []
    for c in range(8):
        b, kv = c // 4, c % 4
        maps.append({
            "xt": xt[b], "wq": wq[kv], "wk": wk[kv], "wv": wv[kv],
            "wo": wo[kv], "cosT": cosT, "sinT": sinT, "tri": tri,
            "perm": perm,
        })
    return maps


def _gather(results):
    out = np.empty((B, S, E), dtype=np.float32)
    for b in range(B):
        acc = results[4 * b]["y"].astype(np.float32)
        for kv in range(1, 4):
            acc += results[4 * b + kv]["y"].astype(np.float32)
        out[b] = acc
    return out


def run(x, Wq, Wk, Wv, Wo, trace=False, **trace_kwargs):
    from concourse.bass_utils import run_bass_kernel_spmd
    nc = _build()
    res = run_bass_kernel_spmd(nc, _in_maps(x, Wq, Wk, Wv, Wo),
                               list(range(8)), trace=trace, **trace_kwargs)
    return _gather(res.results), res


def kernel(x, Wq, Wk, Wv, Wo):
    out, _ = run(np.asarray(x), np.asarray(Wq), np.asarray(Wk),
                 np.asarray(Wv), np.asarray(Wo))
    return out
